# revision 30
# baseline (speedup 1.0000x reference)
"""AttentionDecoder (Bahdanau attention + 2-layer LSTM, T=64 steps) on 8 TRN2
NeuronCores. Data-parallel over batch: each core owns 8 of the 64 batch rows;
all weights replicated; the T-step recurrence runs fully on-chip per core with
no collectives.

v2: col-packed PE matmuls (4 concurrent column-groups via PSUM base-partition
32*g -> tile_position auto-derive), LSTM elementwise in transposed [128,x]
layout (gates transposed via PE; one tanh(0.5*x) ACT instruction covers all 4
gates with the g-gate weight rows pre-doubled host-side; gate order i,f,o,g),
states kept transposed [128,32] so LSTM outputs are directly the stationary
operands of the next matmuls. bf16 matmul operands, fp32 PSUM accumulation,
fp32 cell states.

Self-contained: hardcodes B=64, S=512, H=512, T=64, 8 cores.
"""
import sys
sys.path.insert(0, '/opt/trn_rl_repo')

import numpy as np
import ml_dtypes

import concourse.bass as bass
from concourse import bacc
import concourse.mybir as mybir
from concourse.tile import TileContext
from concourse.bass_utils import run_bass_kernel_spmd
from concourse.masks import make_identity

BF16 = mybir.dt.bfloat16
F32 = mybir.dt.float32
AF = mybir.ActivationFunctionType
OP = mybir.AluOpType
nbf16 = ml_dtypes.bfloat16

N_CORES = 8
B, S, H, T = 64, 512, 512, 64
BL = B // N_CORES          # 8 local batch rows
G4 = 4 * H                 # 2048 gate rows
P = 128

_cache = {}
_last_result = None

# SIM_SEQ=True emits per-group sequential PSUM accumulation brackets (passes
# CoreSim's group checks; slower). False emits one bracket per bank with the
# groups' matmuls interleaved so the 4 PE column-groups stream concurrently
# (has_written is cleared bank-wide once by the single start; each element's
# first write overwrites, so per-element semantics make this exact on HW).
SIM_SEQ = False
DEBUG_T0 = False
GPSIMD_QADD = False


def build(has_b0, has_b1, has_ba):
    nc = bacc.Bacc("TRN2", target_bir_lowering=False, debug=False,
                   num_devices=N_CORES)

    dp = lambda name, shape, dt: nc.dram_tensor(
        name, shape, dt, kind="ExternalInput").ap()

    enc_s = dp("enc_s", [P, BL * 4 * S], BF16)      # (p,(b,st,h)) s on partitions
    enc_t = dp("enc_t", [BL * 4, P, S], BF16)       # ((b,et),p,s) h on partitions
    wih0T = dp("wih0T", [P, 8 * G4], BF16)          # (p,(kt8,g*H+n))
    whh0T = dp("whh0T", [P, 4 * G4], BF16)
    wih1T = dp("wih1T", [P, 4 * G4], BF16)
    whh1T = dp("whh1T", [P, 4 * G4], BF16)
    wahT = dp("wahT", [P, 4 * H], BF16)             # (p,(kt4,ho))
    waeT = dp("waeT", [P, 4 * H], BF16)             # (p,(et4,h))
    wfT = dp("wfT", [P, 4], BF16)
    vblk = dp("vblk", [P, 256], BF16)               # block-diag v
    mskC = dp("mskC", [P, 8], BF16)                 # sum-4-col-groups mask
    selT = dp("selT", [P, 32], BF16)                # gather-col-groups selector
    h00T = dp("h00T", [P, 32], BF16)                # transposed states
    c00T = dp("c00T", [P, 32], F32)
    h01T = dp("h01T", [P, 32], BF16)
    c01T = dp("c01T", [P, 32], F32)
    if has_ba:
        baRow = dp("baRow", [1, H], BF16)
    if has_b0:
        b0Row = dp("b0Row", [1, G4], BF16)
    if has_b1:
        b1Row = dp("b1Row", [1, G4], BF16)
    out = nc.dram_tensor("out", [BL, T], F32, kind="ExternalOutput").ap()
    if DEBUG_T0:
        dbg = {k: nc.dram_tensor("dbg_" + k, shp, F32,
                                 kind="ExternalOutput").ap()
               for k, shp in [("qba", [P, 32]), ("scf", [BL, S]),
                              ("wn", [BL, S]), ("ctxT", [P, 32]),
                              ("t0", [P, P]), ("h0T", [P, 32]),
                              ("h1T", [P, 32]), ("c0T", [P, 32])]}

        def dump(key, ap):
            tmp = cp.tile(ap.shape, F32, name="dbgt_" + key)
            nc.vector.tensor_copy(tmp[:], ap)
            nc.sync.dma_start(dbg[key], tmp[:])
    else:
        dump = None

    with TileContext(nc) as tc:
        with (
            tc.tile_pool(name="const", bufs=1) as cp,
            tc.tile_pool(name="etile", bufs=3) as epool,
            tc.tile_pool(name="ring", bufs=3) as ring,
            tc.tile_pool(name="work", bufs=1) as wk,
            tc.tile_pool(name="work2", bufs=2) as wk2,
            tc.tile_pool(name="ht", bufs=3) as htp,
            tc.tile_pool(name="psBig", bufs=4, space="PSUM") as psB,
            tc.tile_pool(name="psF", bufs=2, space="PSUM") as psF,
            tc.tile_pool(name="psS", bufs=2, space="PSUM") as psS,
        ):
            # ---------------- resident SBUF ----------------
            enc_sb = cp.tile([P, BL * 4 * S], BF16)
            nc.sync.dma_start(enc_sb[:], enc_s)
            wih0_sb = cp.tile([P, 8 * G4], BF16)
            nc.sync.dma_start(wih0_sb[:], wih0T)
            whh0_sb = cp.tile([P, 4 * G4], BF16)
            nc.sync.dma_start(whh0_sb[:], whh0T)
            wih1_sb = cp.tile([P, 4 * G4], BF16)
            nc.sync.dma_start(wih1_sb[:], wih1T)
            whh1_sb = cp.tile([P, 4 * G4], BF16)
            nc.sync.dma_start(whh1_sb[:], whh1T)
            wah_sb = cp.tile([P, 4 * H], BF16)
            nc.sync.dma_start(wah_sb[:], wahT)
            wf_sb = cp.tile([P, 4], BF16)
            nc.sync.dma_start(wf_sb[:], wfT)
            vblk_sb = cp.tile([P, 256], BF16)
            nc.sync.dma_start(vblk_sb[:], vblk)
            msk_sb = cp.tile([P, 8], BF16)
            nc.sync.dma_start(msk_sb[:], mskC)
            selT_sb = cp.tile([P, 32], BF16)
            nc.sync.dma_start(selT_sb[:], selT)
            if has_ba:
                ba_sb = cp.tile([1, H], BF16)
                nc.sync.dma_start(ba_sb[:], baRow)
            if has_b0:
                b0_sb = cp.tile([1, G4], BF16)
                nc.sync.dma_start(b0_sb[:], b0Row)
            if has_b1:
                b1_sb = cp.tile([1, G4], BF16)
                nc.sync.dma_start(b1_sb[:], b1Row)

            ident = cp.tile([P, P], BF16)
            make_identity(nc, ident[:])
            ones18 = cp.tile([1, 8], BF16)
            nc.vector.memset(ones18[:], 1.0)
            wblk_sb = cp.tile([P, 264], BF16)
            nc.vector.memset(wblk_sb[:], 0.0)
            ep_sb = cp.tile([P, BL * 4 * S], BF16)   # (p,(ht,b,s))
            y_acc = cp.tile([BL, T], F32)

            # ---------------- setup: ep = enc @ Wa_e.T (transposed layout) ---
            wae_sb = epool.tile([P, 4 * H], BF16, tag="wae")
            nc.sync.dma_start(wae_sb[:], waeT)
            for b in range(BL):
                eps = [psB.tile([P, S], F32, tag="psB", name=f"eps{_i}")
                       for _i in range(4)]
                for et in range(4):
                    et_sb = ring.tile([P, S], BF16, tag="encT")
                    nc.sync.dma_start(et_sb[:], enc_t[b * 4 + et])
                    for ht in range(4):
                        nc.tensor.matmul(
                            eps[ht][:],
                            wae_sb[:, et * H + ht * P: et * H + (ht + 1) * P],
                            et_sb[:],
                            start=(et == 0), stop=(et == 3))
                for ht in range(4):
                    nc.scalar.copy(
                        ep_sb[:, (ht * BL + b) * S:(ht * BL + b + 1) * S],
                        eps[ht][:])

            # one-time PSUM bank initialization: psS/psF banks may hold
            # junk (NaN bit patterns) at power-on; full-tile CAST copies and
            # selector matmuls read all 128 partitions, so scrub them once.
            if not SIM_SEQ:
                for _i in range(2):
                    scrub = psS.tile([P, S], F32, tag="psS", name=f"scr{_i}")
                    nc.vector.memset(scrub[:], 0.0)
                for _i in range(2):
                    scrub = psF.tile([P, S], F32, tag="ps8", name=f"scf{_i}")
                    nc.vector.memset(scrub[:], 0.0)

            # ---------------- setup: states (host-transposed) ----------------
            c0T = cp.tile([P, 32], F32)
            nc.sync.dma_start(c0T[:], c00T)
            c1T = cp.tile([P, 32], F32)
            nc.sync.dma_start(c1T[:], c01T)
            h0T = htp.tile([P, 32], BF16, tag="h0T")
            nc.sync.dma_start(h0T[:], h00T)
            h1T = htp.tile([P, 32], BF16, tag="h1T")
            nc.sync.dma_start(h1T[:], h01T)

            def emit_qpath(h1T_):
                """q = h1 @ Wa_h.T (+ba), produced as transposed [128,32] f32.
                Col-packed 4 N-quarter groups; group j lands at partitions
                32j so its transpose is a row-group-j PE transpose."""
                q_ps = psS.tile([P, P], F32, tag="psS")
                for j in range(4):
                    n_mm = 4 + (1 if has_ba else 0)
                    for kt in range(4):
                        nc.tensor.matmul(
                            q_ps[32 * j:32 * j + 8, :],
                            h1T_[:, 8 * kt:8 * kt + 8],
                            wah_sb[:, kt * H + 128 * j:kt * H + 128 * (j + 1)],
                            start=(kt == 0), stop=(kt == n_mm - 1),
                            tile_position=(0, 32 * j))
                    if has_ba:
                        nc.tensor.matmul(q_ps[32 * j:32 * j + 8, :],
                                         ones18[:],
                                         ba_sb[:, 128 * j:128 * (j + 1)],
                                         start=False, stop=True,
                                         tile_position=(0, 32 * j))
                q_sb = wk2.tile([P, P], BF16, tag="q_sb")
                if SIM_SEQ:
                    nc.vector.memset(q_sb[:], 0.0)
                    for j in range(4):
                        nc.vector.tensor_copy(q_sb[32 * j:32 * j + 8, :],
                                              q_ps[32 * j:32 * j + 8, :])
                else:
                    nc.vector.tensor_copy(q_sb[:], q_ps[:])
                # transpose+gather all 4 col-groups with one full-array
                # matmul: out[m, 8j+b] = q_sb[32j+b, m]
                qT_ps = psS.tile([P, 32], F32, tag="psS")
                nc.tensor.matmul(qT_ps[:], q_sb[:], selT_sb[:],
                                 start=True, stop=True)
                qdst = wk2.tile([P, 32], F32, tag="qbaT")
                nc.vector.tensor_copy(qdst[:], qT_ps[:])
                return qdst

            qbaT = emit_qpath(h1T)
            if DEBUG_T0:
                dump("qba", qbaT[:])

            def gate_mm(gps_, gg, lhsT_, rhs_, start, stop):
                nc.tensor.matmul(
                    gps_[32 * gg:32 * gg + 8, :], lhsT_, rhs_,
                    start=start, stop=stop, tile_position=(0, 32 * gg),
                    skip_group_check=not SIM_SEQ)

            def emit_gates0_hdec(gps_, h0T_, decT_):
                """h- and dec- parts of layer-0 gates, col-packed: gate gg at
                partitions 32gg. Interleaved kt-major (HW mode) so the 4
                column-groups stream concurrently; single bank bracket opens
                here (start on the very first matmul only). decT_=None at
                t=0 (the reference's dec_in starts as zeros)."""
                order = ([(gg, kt) for gg in range(4) for kt in range(4)]
                         if SIM_SEQ else
                         [(gg, kt) for kt in range(4) for gg in range(4)])
                for gg, kt in order:
                    gate_mm(gps_, gg,
                            h0T_[:, 8 * kt:8 * kt + 8],
                            whh0_sb[:, kt * G4 + gg * H:kt * G4 + (gg + 1) * H],
                            kt == 0, False)
                    if has_b0 and kt == 3:
                        gate_mm(gps_, gg, ones18[:],
                                b0_sb[:, gg * H:(gg + 1) * H], False, False)
                if decT_ is not None:
                    for gg, kt in order:
                        gate_mm(gps_, gg,
                                decT_[:, 8 * kt:8 * kt + 8],
                                wih0_sb[:, kt * G4 + gg * H:
                                        kt * G4 + (gg + 1) * H],
                                False, False)

            def emit_gates1_h(gps_, h1T_):
                order = ([(gg, kt) for gg in range(4) for kt in range(4)]
                         if SIM_SEQ else
                         [(gg, kt) for kt in range(4) for gg in range(4)])
                for gg, kt in order:
                    gate_mm(gps_, gg,
                            h1T_[:, 8 * kt:8 * kt + 8],
                            whh1_sb[:, kt * G4 + gg * H:kt * G4 + (gg + 1) * H],
                            kt == 0, False)
                    if has_b1 and kt == 3:
                        gate_mm(gps_, gg, ones18[:],
                                b1_sb[:, gg * H:(gg + 1) * H], False, False)

            def lstm_elementwise(g_ps, g_tag, cT, h_tag):
                """Gates PSUM [128(4 gate-groups at 32*gg),512] -> transposed
                [128,128] -> one tanh(0.5*x) ACT (g-gate weights pre-doubled,
                gate order i,f,o,g) -> h^T [128,32] bf16 (stationary form)."""
                g_sb = wk2.tile([P, S], BF16, tag=g_tag)
                if SIM_SEQ:
                    nc.vector.memset(g_sb[:], 0.0)
                    for gg in range(4):
                        nc.vector.tensor_copy(g_sb[32 * gg:32 * gg + 8, :],
                                              g_ps[32 * gg:32 * gg + 8, :])
                else:
                    pass
                # per h-tile selector matmul (contiguous out): col layout
                # kt*32 + gg*8 + b ; out[m, 8gg+b] = g_sb[32gg+b, kt*128+m].
                # HW mode: quartered cast interleaved with the sel-MMs.
                gT_ps = psS.tile([P, P], F32, tag="psS")
                for kt in range(4):
                    if not SIM_SEQ:
                        nc.vector.tensor_copy(g_sb[:, kt * P:(kt + 1) * P],
                                              g_ps[:, kt * P:(kt + 1) * P])
                    nc.tensor.matmul(gT_ps[:, kt * 32:(kt + 1) * 32],
                                     g_sb[:, kt * P:(kt + 1) * P],
                                     selT_sb[:], start=True, stop=True)
                t_sb = wk.tile([P, P], BF16, tag="t_" + g_tag)
                nc.scalar.activation(t_sb[:], gT_ps[:], AF.Tanh, scale=0.5)
                # t cols: (kt, gate, b) with gate order i,f,o,g
                t_v = t_sb[:].rearrange("p (k g b) -> p k g b", k=4, g=4, b=8)
                s_sb = wk.tile([P, 96], BF16, tag="s_" + g_tag)
                s_v = s_sb[:].rearrange("p (k g b) -> p k g b", k=4, g=3, b=8)
                nc.vector.tensor_scalar(s_v[:], t_v[:, :, 0:3, :], 0.5, 0.5,
                                        op0=OP.mult, op1=OP.add)
                cT_v = cT[:].rearrange("p (k b) -> p k b", k=4)
                t1 = wk.tile([P, 32], F32, tag="t1")
                t1_v = t1[:].rearrange("p (k b) -> p k b", k=4)
                nc.vector.tensor_mul(t1_v[:], s_v[:, :, 0, :],
                                     t_v[:, :, 3, :])
                t2 = wk.tile([P, 32], F32, tag="t2")
                t2_v = t2[:].rearrange("p (k b) -> p k b", k=4)
                nc.vector.tensor_mul(t2_v[:], s_v[:, :, 1, :], cT_v[:])
                nc.vector.tensor_add(cT[:], t2[:], t1[:])
                nc.tensor.ldweights(ident[:])   # HAM keep-warm
                tc2 = wk.tile([P, 32], BF16, tag="tc_" + g_tag)
                nc.scalar.activation(tc2[:], cT[:], AF.Tanh)
                hT = htp.tile([P, 32], BF16, tag=h_tag)
                hT_v = hT[:].rearrange("p (k b) -> p k b", k=4)
                tc2_v = tc2[:].rearrange("p (k b) -> p k b", k=4)
                nc.vector.tensor_mul(hT_v[:], s_v[:, :, 2, :], tc2_v[:])
                return hT

            # ---------------- the recurrence ----------------
            for t in range(T):
                # early PE fodder: all gate matmuls that only need step-entry
                # state (they run while ACT chews the energy tanh). In SIM_SEQ
                # mode brackets must be contiguous, so these move to the end.
                g0_ps = psB.tile([P, S], F32, tag="psB", name="g0")
                g1_ps = psB.tile([P, S], F32, tag="psB", name="g1")
                if not SIM_SEQ:
                    emit_gates0_hdec(g0_ps, h0T, h1T if t > 0 else None)
                    emit_gates1_h(g1_ps, h1T)

                # ---- attention: energy tanh + scores ----
                # scores stay unpacked: the 32 serial matmuls hide under the
                # tanh chain, and the softmax can then read PSUM directly.
                scores_ps = psB.tile([BL, S], F32, tag="psB", name="sc")
                for ht in range(4):
                    e_t = epool.tile([P, BL * S], BF16, tag="e")
                    for b in range(BL):
                        eng = nc.gpsimd if (GPSIMD_QADD and b >= 5) else nc.vector
                        eng.tensor_scalar(
                            e_t[:, b * S:(b + 1) * S],
                            ep_sb[:, (ht * BL + b) * S:(ht * BL + b + 1) * S],
                            qbaT[:, 8 * ht + b:8 * ht + b + 1], None,
                            op0=OP.add)
                    if ht == 0:
                        # first tanh of the step is on the critical path:
                        # split it so it starts after only 4 q-adds
                        nc.scalar.activation(e_t[:, 0:4 * S], e_t[:, 0:4 * S],
                                             AF.Tanh)
                        nc.scalar.activation(e_t[:, 4 * S:], e_t[:, 4 * S:],
                                             AF.Tanh)
                    else:
                        nc.scalar.activation(e_t[:], e_t[:], AF.Tanh)
                    for b in range(BL):
                        kt = ht * 8 + b
                        nc.tensor.matmul(
                            scores_ps[:],
                            vblk_sb[:, 8 * kt:8 * kt + 8],
                            e_t[:, b * S:(b + 1) * S],
                            start=(kt == 0), stop=(kt == 31))

                # ---- softmax (reads scores PSUM directly) ----
                w_e = wk.tile([BL, S], BF16, tag="w_e")
                zsum = wk.tile([BL, 1], F32, tag="zsum")
                nc.scalar.activation(w_e[:], scores_ps[:], AF.Exp,
                                     accum_out=zsum[:])
                zrec = wk.tile([BL, 1], F32, tag="zrec")
                nc.vector.reciprocal(zrec[:], zsum[:])
                nc.tensor.ldweights(ident[:])   # HAM keep-warm
                w_n = wk.tile([BL, S], BF16, tag="w_n")
                nc.vector.tensor_scalar(w_n[:], w_e[:], zrec[:, 0:1], None,
                                        op0=OP.mult)
                if DEBUG_T0 and t == 0:
                    dump("scf", scores_f[:])
                    dump("wn", w_n[:])

                # ---- w transpose -> block-diag stationary ----
                wT_ps = psS.tile([P, 32], BF16, tag="psS")
                for j in range(4):
                    nc.tensor.matmul(wT_ps[:, 8 * j:8 * j + 8],
                                     w_n[:, 128 * j:128 * (j + 1)],
                                     ident[:8, :8], is_transpose=True)
                wblk_v = wblk_sb[:, 0:264].rearrange("p (b c) -> p b c",
                                                      c=33)
                wT_v = wT_ps[:].rearrange("p (s b) -> p b s", b=8)
                nc.vector.tensor_copy(wblk_v[:, :, 0:25:8], wT_v[:])

                # ---- context (col-packed by batch-pair) ----
                ctx_ps = psB.tile([P, S], F32, tag="psB", name="ctx")
                if SIM_SEQ:
                    order = [(g, r) for g in range(4) for r in range(8)]
                else:
                    order = [(g, r) for r in range(8) for g in range(4)]
                for g, r in order:
                    b = 2 * g + r // 4
                    st = r % 4
                    kt = b * 4 + st
                    first = (r == 0)
                    last = (r == 7)
                    nc.tensor.matmul(
                        ctx_ps[32 * g:32 * g + 8, :],
                        wblk_sb[:, 8 * kt:8 * kt + 8],
                        enc_sb[:, kt * S:(kt + 1) * S],
                        start=first, stop=last,
                        tile_position=(0, 32 * g),
                        skip_group_check=not SIM_SEQ)
                ctx_sb = wk2.tile([P, S], BF16, tag="ctx_sb")
                if SIM_SEQ:
                    nc.vector.memset(ctx_sb[:], 0.0)
                    for g in range(4):
                        nc.vector.tensor_copy(ctx_sb[32 * g:32 * g + 8, :],
                                              ctx_ps[32 * g:32 * g + 8, :])
                else:
                    pass
                # transpose + sum the 4 partials: out[m, 8kt+b] =
                # sum_g ctx_sb[32g+b, kt*128+m]. In HW mode the cast is
                # quartered and interleaved so cast kt+1 overlaps sel-MM kt.
                ctxT_ps = psS.tile([P, 32], F32, tag="psS")
                for kt in range(4):
                    if not SIM_SEQ:
                        nc.vector.tensor_copy(ctx_sb[:, kt * P:(kt + 1) * P],
                                              ctx_ps[:, kt * P:(kt + 1) * P])
                    nc.tensor.matmul(ctxT_ps[:, 8 * kt:8 * kt + 8],
                                     ctx_sb[:, kt * P:(kt + 1) * P],
                                     msk_sb[:], start=True, stop=True)
                ctxT_sb = wk2.tile([P, 32], BF16, tag="ctxT_sb")
                nc.vector.tensor_copy(ctxT_sb[:], ctxT_ps[:])
                if DEBUG_T0 and t == 0:
                    dump("ctxT", ctxT_sb[:])

                # ---- finish gates0: ctx part (koffset 4 in wih0) ----
                if SIM_SEQ:
                    def g0_bracket(gg):
                        for kt in range(4):
                            gate_mm(g0_ps, gg, h0T[:, 8 * kt:8 * kt + 8],
                                    whh0_sb[:, kt * G4 + gg * H:
                                            kt * G4 + (gg + 1) * H],
                                    kt == 0, False)
                        if has_b0:
                            gate_mm(g0_ps, gg, ones18[:],
                                    b0_sb[:, gg * H:(gg + 1) * H], False, False)
                        if t > 0:
                            for kt in range(4):
                                gate_mm(g0_ps, gg, h1T[:, 8 * kt:8 * kt + 8],
                                        wih0_sb[:, kt * G4 + gg * H:
                                                kt * G4 + (gg + 1) * H],
                                        False, False)
                        for kt in range(4):
                            gate_mm(g0_ps, gg, ctxT_sb[:, 8 * kt:8 * kt + 8],
                                    wih0_sb[:, (4 + kt) * G4 + gg * H:
                                            (4 + kt) * G4 + (gg + 1) * H],
                                    False, kt == 3)
                    for gg in range(4):
                        g0_bracket(gg)
                else:
                    order = [(gg, kt) for kt in range(4) for gg in range(4)]
                    for gg, kt in order:
                        gate_mm(g0_ps, gg, ctxT_sb[:, 8 * kt:8 * kt + 8],
                                wih0_sb[:, (4 + kt) * G4 + gg * H:
                                        (4 + kt) * G4 + (gg + 1) * H],
                                False, kt == 3)

                h0T = lstm_elementwise(g0_ps, "g0s", c0T, "h0T")
                if DEBUG_T0 and t == 0:
                    dump("h0T", h0T[:])
                    dump("c0T", c0T[:])

                # ---- LSTM layer 1: x part ----
                if SIM_SEQ:
                    for gg in range(4):
                        for kt in range(4):
                            gate_mm(g1_ps, gg, h1T[:, 8 * kt:8 * kt + 8],
                                    whh1_sb[:, kt * G4 + gg * H:
                                            kt * G4 + (gg + 1) * H],
                                    kt == 0, False)
                        if has_b1:
                            gate_mm(g1_ps, gg, ones18[:],
                                    b1_sb[:, gg * H:(gg + 1) * H], False, False)
                        for kt in range(4):
                            gate_mm(g1_ps, gg, h0T[:, 8 * kt:8 * kt + 8],
                                    wih1_sb[:, kt * G4 + gg * H:
                                            kt * G4 + (gg + 1) * H],
                                    False, kt == 3)
                else:
                    order = [(gg, kt) for kt in range(4) for gg in range(4)]
                    for gg, kt in order:
                        gate_mm(g1_ps, gg, h0T[:, 8 * kt:8 * kt + 8],
                                wih1_sb[:, kt * G4 + gg * H:
                                        kt * G4 + (gg + 1) * H],
                                False, kt == 3)
                h1T = lstm_elementwise(g1_ps, "g1s", c1T, "h1T")
                if DEBUG_T0 and t == 0:
                    dump("h1T", h1T[:])

                # ---- qba for next step first (unblocks next tanh ASAP) ----
                if t + 1 < T:
                    qbaT = emit_qpath(h1T)

                # ---- y = h1n @ Wf.T ----
                y_ps = psF.tile([BL, 1], F32, tag="ps8")
                for kt in range(4):
                    nc.tensor.matmul(y_ps[:], h1T[:, 8 * kt:8 * kt + 8],
                                     wf_sb[:, kt:kt + 1],
                                     start=(kt == 0), stop=(kt == 3))
                nc.vector.tensor_copy(y_acc[:, t:t + 1], y_ps[:])

            nc.sync.dma_start(out[:], y_acc[:])

    nc.compile()
    return nc


def _marshal(inputs):
    """Host-side shard + relayout. Returns (in_maps, flags)."""
    f32 = np.float32
    enc = np.asarray(inputs["encoder_outputs"], f32)
    Wa_h = np.asarray(inputs["Wa_h"], f32)
    Wa_e = np.asarray(inputs["Wa_e"], f32)
    ba = np.asarray(inputs["ba"], f32)
    v = np.asarray(inputs["v"], f32)
    W_ih_0 = np.asarray(inputs["W_ih_0"], f32)
    W_hh_0 = np.asarray(inputs["W_hh_0"], f32)
    b_0 = np.asarray(inputs["b_0"], f32)
    W_ih_1 = np.asarray(inputs["W_ih_1"], f32)
    W_hh_1 = np.asarray(inputs["W_hh_1"], f32)
    b_1 = np.asarray(inputs["b_1"], f32)
    Wf = np.asarray(inputs["Wf"], f32)

    has_ba = bool(np.any(ba != 0))
    has_b0 = bool(np.any(b_0 != 0))
    has_b1 = bool(np.any(b_1 != 0))

    def gate_perm(W):
        # torch gate order i,f,g,o -> i,f,o,2*g (one tanh(0.5*x) ACT serves
        # sigmoid-as-tanh for i,f,o and plain tanh for the doubled g rows)
        return np.concatenate([W[0:H], W[H:2 * H], W[3 * H:4 * H],
                               2.0 * W[2 * H:3 * H]], axis=0)

    W_ih_0 = gate_perm(W_ih_0)
    W_hh_0 = gate_perm(W_hh_0)
    W_ih_1 = gate_perm(W_ih_1)
    W_hh_1 = gate_perm(W_hh_1)
    b_0 = gate_perm(b_0.reshape(G4, 1)).reshape(G4)
    b_1 = gate_perm(b_1.reshape(G4, 1)).reshape(G4)

    def to_kxn(W, n_kt):  # W [N, K] -> [128, n_kt*N] bf16 layout (p,(kt,n))
        Wt = W.T.astype(nbf16)                       # [K, N]
        return np.ascontiguousarray(
            Wt.reshape(n_kt, P, W.shape[0]).transpose(1, 0, 2).reshape(P, -1))

    def to_T32(x, dt):  # [8, 512] -> [128, 32] col kt*8+b
        return np.ascontiguousarray(
            x.T.reshape(4, P, BL).transpose(1, 0, 2).reshape(P, 32).astype(dt))

    wih0T = to_kxn(W_ih_0, 8)
    whh0T = to_kxn(W_hh_0, 4)
    wih1T = to_kxn(W_ih_1, 4)
    whh1T = to_kxn(W_hh_1, 4)
    wahT = to_kxn(Wa_h, 4)
    waeT = to_kxn(Wa_e, 4)
    wfT = np.ascontiguousarray(Wf.reshape(4, P).T.astype(nbf16))  # [128,4]

    vblk = np.zeros((P, 256), nbf16)
    for ht in range(4):
        for b in range(BL):
            kt = ht * 8 + b
            vblk[:, 8 * kt + b] = v[ht * P:(ht + 1) * P].astype(nbf16)

    mskC = np.zeros((P, 8), nbf16)
    selT = np.zeros((P, 32), nbf16)
    for g in range(4):
        for b in range(BL):
            mskC[32 * g + b, b] = 1.0
            selT[32 * g + b, 8 * g + b] = 1.0

    shared = dict(wih0T=wih0T, whh0T=whh0T, wih1T=wih1T, whh1T=whh1T,
                  wahT=wahT, waeT=waeT, wfT=wfT, vblk=vblk,
                  mskC=mskC, selT=selT)
    if has_ba:
        shared["baRow"] = ba.reshape(1, H).astype(nbf16)
    if has_b0:
        shared["b0Row"] = b_0.reshape(1, G4).astype(nbf16)
    if has_b1:
        shared["b1Row"] = b_1.reshape(1, G4).astype(nbf16)

    enc_bf = enc.astype(nbf16)
    in_maps = []
    for c in range(N_CORES):
        sl = slice(c * BL, (c + 1) * BL)
        eb = enc_bf[sl]                                   # [8, 512, 512]
        enc_s = np.ascontiguousarray(
            eb.reshape(BL, 4, P, H).transpose(2, 0, 1, 3).reshape(P, -1))
        enc_t = np.ascontiguousarray(
            eb.transpose(0, 2, 1).reshape(BL, 4, P, S).reshape(BL * 4, P, S))
        m = dict(shared)
        m.update(
            enc_s=enc_s, enc_t=enc_t,
            h00T=to_T32(np.asarray(inputs["h0_0"], f32)[sl], nbf16),
            c00T=to_T32(np.asarray(inputs["c0_0"], f32)[sl], f32),
            h01T=to_T32(np.asarray(inputs["h0_1"], f32)[sl], nbf16),
            c01T=to_T32(np.asarray(inputs["c0_1"], f32)[sl], f32),
        )
        in_maps.append(m)
    return in_maps, (has_b0, has_b1, has_ba)


def kernel(**inputs):
    global _last_result
    in_maps, flags = _marshal(inputs)
    if flags not in _cache:
        _cache[flags] = build(*flags)
    nc = _cache[flags]
    res = run_bass_kernel_spmd(nc, in_maps, core_ids=list(range(N_CORES)))
    _last_result = res
    ys = np.concatenate([np.asarray(res.results[i]["out"], np.float32)
                         for i in range(N_CORES)], axis=0)   # [64, 64]
    bf_ = np.asarray(inputs["bf"], np.float32).reshape(1, 1)
    y = ys + bf_
    return y.reshape(B, T, 1).astype(np.float32)


if __name__ == "__main__":
    rng = np.random.default_rng(0)
    fake = {
        "encoder_outputs": rng.normal(size=(B, S, H)).astype(np.float32),
        "h0_0": rng.normal(size=(B, H)).astype(np.float32),
        "c0_0": rng.normal(size=(B, H)).astype(np.float32),
        "h0_1": rng.normal(size=(B, H)).astype(np.float32),
        "c0_1": rng.normal(size=(B, H)).astype(np.float32),
        "Wa_h": (rng.normal(size=(H, H)) * 0.05).astype(np.float32),
        "Wa_e": (rng.normal(size=(H, H)) * 0.05).astype(np.float32),
        "ba": np.zeros(H, np.float32),
        "v": (rng.normal(size=H) * 0.05).astype(np.float32),
        "W_ih_0": (rng.normal(size=(G4, 2 * H)) * 0.05).astype(np.float32),
        "W_hh_0": (rng.normal(size=(G4, H)) * 0.05).astype(np.float32),
        "b_0": np.zeros(G4, np.float32),
        "W_ih_1": (rng.normal(size=(G4, H)) * 0.05).astype(np.float32),
        "W_hh_1": (rng.normal(size=(G4, H)) * 0.05).astype(np.float32),
        "b_1": np.zeros(G4, np.float32),
        "Wf": (rng.normal(size=(1, H)) * 0.05).astype(np.float32),
        "bf": np.zeros(1, np.float32),
    }
    y = kernel(**fake)
    print("kernel output", y.shape, y.dtype, float(np.abs(y).max()))


# revision 31
# speedup vs baseline: 1.3064x; 1.3064x over previous
"""AttentionDecoder (Bahdanau attention + 2-layer LSTM, T=64 steps) on 8 TRN2
NeuronCores. Data-parallel over batch: each core owns 8 of the 64 batch rows;
all weights replicated; the T-step recurrence runs fully on-chip per core with
no collectives.

v2: col-packed PE matmuls (4 concurrent column-groups via PSUM base-partition
32*g -> tile_position auto-derive), LSTM elementwise in transposed [128,x]
layout (gates transposed via PE; one tanh(0.5*x) ACT instruction covers all 4
gates with the g-gate weight rows pre-doubled host-side; gate order i,f,o,g),
states kept transposed [128,32] so LSTM outputs are directly the stationary
operands of the next matmuls. bf16 matmul operands, fp32 PSUM accumulation,
fp32 cell states.

Self-contained: hardcodes B=64, S=512, H=512, T=64, 8 cores.
"""
import sys
sys.path.insert(0, '/opt/trn_rl_repo')

import numpy as np
import ml_dtypes

import concourse.bass as bass
from concourse import bacc
import concourse.mybir as mybir
from concourse.tile import TileContext
from concourse.bass_utils import run_bass_kernel_spmd
from concourse.masks import make_identity

BF16 = mybir.dt.bfloat16
F32 = mybir.dt.float32
AF = mybir.ActivationFunctionType
OP = mybir.AluOpType
nbf16 = ml_dtypes.bfloat16

N_CORES = 8
B, S, H, T = 64, 512, 512, 64
BL = B // N_CORES          # 8 local batch rows
G4 = 4 * H                 # 2048 gate rows
P = 128

_cache = {}
_last_result = None

# SIM_SEQ=True emits per-group sequential PSUM accumulation brackets (passes
# CoreSim's group checks; slower). False emits one bracket per bank with the
# groups' matmuls interleaved so the 4 PE column-groups stream concurrently
# (has_written is cleared bank-wide once by the single start; each element's
# first write overwrites, so per-element semantics make this exact on HW).
SIM_SEQ = False
DEBUG_T0 = False
GPSIMD_QADD = False


def build(has_b0, has_b1, has_ba):
    nc = bacc.Bacc("TRN2", target_bir_lowering=False, debug=False,
                   num_devices=N_CORES)

    dp = lambda name, shape, dt: nc.dram_tensor(
        name, shape, dt, kind="ExternalInput").ap()

    enc_s = dp("enc_s", [P, BL * 4 * S], BF16)      # (p,(b,st,h)) s on partitions
    enc_t = dp("enc_t", [BL * 4, P, S], BF16)       # ((b,et),p,s) h on partitions
    wih0T = dp("wih0T", [P, 8 * G4], BF16)          # (p,(kt8,g*H+n))
    whh0T = dp("whh0T", [P, 4 * G4], BF16)
    wih1T = dp("wih1T", [P, 4 * G4], BF16)
    whh1T = dp("whh1T", [P, 4 * G4], BF16)
    wahT = dp("wahT", [P, 4 * H], BF16)             # (p,(kt4,ho))
    waeT = dp("waeT", [P, 4 * H], BF16)             # (p,(et4,h))
    wfT = dp("wfT", [P, 4], BF16)
    vblk = dp("vblk", [P, 256], BF16)               # block-diag v
    mskC = dp("mskC", [P, 8], BF16)                 # sum-4-col-groups mask
    selT = dp("selT", [P, 32], BF16)                # gather-col-groups selector
    h00T = dp("h00T", [P, 32], BF16)                # transposed states
    c00T = dp("c00T", [P, 32], F32)
    h01T = dp("h01T", [P, 32], BF16)
    c01T = dp("c01T", [P, 32], F32)
    if has_ba:
        baRow = dp("baRow", [1, H], BF16)
    if has_b0:
        b0Row = dp("b0Row", [1, G4], BF16)
    if has_b1:
        b1Row = dp("b1Row", [1, G4], BF16)
    out = nc.dram_tensor("out", [BL, T], F32, kind="ExternalOutput").ap()
    if DEBUG_T0:
        dbg = {k: nc.dram_tensor("dbg_" + k, shp, F32,
                                 kind="ExternalOutput").ap()
               for k, shp in [("qba", [P, 32]), ("scf", [BL, S]),
                              ("wn", [BL, S]), ("ctxT", [P, 32]),
                              ("t0", [P, P]), ("h0T", [P, 32]),
                              ("h1T", [P, 32]), ("c0T", [P, 32])]}

        def dump(key, ap):
            tmp = cp.tile(ap.shape, F32, name="dbgt_" + key)
            nc.vector.tensor_copy(tmp[:], ap)
            nc.sync.dma_start(dbg[key], tmp[:])
    else:
        dump = None

    with TileContext(nc) as tc:
        with (
            tc.tile_pool(name="const", bufs=1) as cp,
            tc.tile_pool(name="etile", bufs=3) as epool,
            tc.tile_pool(name="ring", bufs=3) as ring,
            tc.tile_pool(name="work", bufs=1) as wk,
            tc.tile_pool(name="work2", bufs=2) as wk2,
            tc.tile_pool(name="ht", bufs=3) as htp,
            tc.tile_pool(name="psBig", bufs=4, space="PSUM") as psB,
            tc.tile_pool(name="psF", bufs=2, space="PSUM") as psF,
            tc.tile_pool(name="psS", bufs=2, space="PSUM") as psS,
        ):
            # ---------------- resident SBUF ----------------
            enc_sb = cp.tile([P, BL * 4 * S], BF16)
            nc.sync.dma_start(enc_sb[:], enc_s)
            wih0_sb = cp.tile([P, 8 * G4], BF16)
            nc.sync.dma_start(wih0_sb[:], wih0T)
            whh0_sb = cp.tile([P, 4 * G4], BF16)
            nc.sync.dma_start(whh0_sb[:], whh0T)
            wih1_sb = cp.tile([P, 4 * G4], BF16)
            nc.sync.dma_start(wih1_sb[:], wih1T)
            whh1_sb = cp.tile([P, 4 * G4], BF16)
            nc.sync.dma_start(whh1_sb[:], whh1T)
            wah_sb = cp.tile([P, 4 * H], BF16)
            nc.sync.dma_start(wah_sb[:], wahT)
            wf_sb = cp.tile([P, 4], BF16)
            nc.sync.dma_start(wf_sb[:], wfT)
            vblk_sb = cp.tile([P, 256], BF16)
            nc.sync.dma_start(vblk_sb[:], vblk)
            msk_sb = cp.tile([P, 8], BF16)
            nc.sync.dma_start(msk_sb[:], mskC)
            selT_sb = cp.tile([P, 32], BF16)
            nc.sync.dma_start(selT_sb[:], selT)
            if has_ba:
                ba_sb = cp.tile([1, H], BF16)
                nc.sync.dma_start(ba_sb[:], baRow)
            if has_b0:
                b0_sb = cp.tile([1, G4], BF16)
                nc.sync.dma_start(b0_sb[:], b0Row)
            if has_b1:
                b1_sb = cp.tile([1, G4], BF16)
                nc.sync.dma_start(b1_sb[:], b1Row)

            ident = cp.tile([P, P], BF16)
            make_identity(nc, ident[:])
            ones18 = cp.tile([1, 8], BF16)
            nc.vector.memset(ones18[:], 1.0)
            wblk_sb = cp.tile([P, 264], BF16)
            nc.vector.memset(wblk_sb[:], 0.0)
            ep_sb = cp.tile([P, BL * 4 * S], BF16)   # (p,(ht,b,s))
            y_acc = cp.tile([BL, T], F32)

            # ---------------- setup: ep = enc @ Wa_e.T (transposed layout) ---
            wae_sb = epool.tile([P, 4 * H], BF16, tag="wae")
            nc.sync.dma_start(wae_sb[:], waeT)
            for b in range(BL):
                eps = [psB.tile([P, S], F32, tag="psB", name=f"eps{_i}")
                       for _i in range(4)]
                for et in range(4):
                    et_sb = ring.tile([P, S], BF16, tag="encT")
                    nc.sync.dma_start(et_sb[:], enc_t[b * 4 + et])
                    for ht in range(4):
                        nc.tensor.matmul(
                            eps[ht][:],
                            wae_sb[:, et * H + ht * P: et * H + (ht + 1) * P],
                            et_sb[:],
                            start=(et == 0), stop=(et == 3))
                for ht in range(4):
                    nc.scalar.copy(
                        ep_sb[:, (ht * BL + b) * S:(ht * BL + b + 1) * S],
                        eps[ht][:])

            # one-time PSUM bank initialization: psS/psF banks may hold
            # junk (NaN bit patterns) at power-on; full-tile CAST copies and
            # selector matmuls read all 128 partitions, so scrub them once.
            if not SIM_SEQ:
                for _i in range(2):
                    scrub = psS.tile([P, S], F32, tag="psS", name=f"scr{_i}")
                    nc.vector.memset(scrub[:], 0.0)
                for _i in range(2):
                    scrub = psF.tile([P, S], F32, tag="ps8", name=f"scf{_i}")
                    nc.vector.memset(scrub[:], 0.0)

            # ---------------- setup: states (host-transposed) ----------------
            c0T = cp.tile([P, 32], F32)
            nc.sync.dma_start(c0T[:], c00T)
            c1T = cp.tile([P, 32], F32)
            nc.sync.dma_start(c1T[:], c01T)
            h0T = htp.tile([P, 32], BF16, tag="h0T")
            nc.sync.dma_start(h0T[:], h00T)
            h1T = htp.tile([P, 32], BF16, tag="h1T")
            nc.sync.dma_start(h1T[:], h01T)

            def emit_qpath(h1T_):
                """q = h1 @ Wa_h.T (+ba), produced as transposed [128,32] f32.
                Col-packed 4 N-quarter groups; group j lands at partitions
                32j so its transpose is a row-group-j PE transpose."""
                q_ps = psS.tile([P, P], F32, tag="psS")
                for j in range(4):
                    n_mm = 4 + (1 if has_ba else 0)
                    for kt in range(4):
                        nc.tensor.matmul(
                            q_ps[32 * j:32 * j + 8, :],
                            h1T_[:, 8 * kt:8 * kt + 8],
                            wah_sb[:, kt * H + 128 * j:kt * H + 128 * (j + 1)],
                            start=(kt == 0), stop=(kt == n_mm - 1),
                            tile_position=(0, 32 * j))
                    if has_ba:
                        nc.tensor.matmul(q_ps[32 * j:32 * j + 8, :],
                                         ones18[:],
                                         ba_sb[:, 128 * j:128 * (j + 1)],
                                         start=False, stop=True,
                                         tile_position=(0, 32 * j))
                q_sb = wk2.tile([P, P], BF16, tag="q_sb")
                if SIM_SEQ:
                    nc.vector.memset(q_sb[:], 0.0)
                    for j in range(4):
                        nc.vector.tensor_copy(q_sb[32 * j:32 * j + 8, :],
                                              q_ps[32 * j:32 * j + 8, :])
                else:
                    nc.vector.tensor_copy(q_sb[:], q_ps[:])
                # transpose+gather all 4 col-groups with one full-array
                # matmul: out[m, 8j+b] = q_sb[32j+b, m]
                qT_ps = psS.tile([P, 32], F32, tag="psS")
                nc.tensor.matmul(qT_ps[:], q_sb[:], selT_sb[:],
                                 start=True, stop=True)
                qdst = wk2.tile([P, 32], F32, tag="qbaT")
                nc.vector.tensor_copy(qdst[:], qT_ps[:])
                return qdst

            qbaT = emit_qpath(h1T)
            if DEBUG_T0:
                dump("qba", qbaT[:])

            def gate_mm(gps_, gg, lhsT_, rhs_, start, stop):
                nc.tensor.matmul(
                    gps_[32 * gg:32 * gg + 8, :], lhsT_, rhs_,
                    start=start, stop=stop, tile_position=(0, 32 * gg),
                    skip_group_check=not SIM_SEQ)

            def emit_gates0_hdec(gps_, h0T_, decT_):
                """h- and dec- parts of layer-0 gates, col-packed: gate gg at
                partitions 32gg. Interleaved kt-major (HW mode) so the 4
                column-groups stream concurrently; single bank bracket opens
                here (start on the very first matmul only). decT_=None at
                t=0 (the reference's dec_in starts as zeros)."""
                order = ([(gg, kt) for gg in range(4) for kt in range(4)]
                         if SIM_SEQ else
                         [(gg, kt) for kt in range(4) for gg in range(4)])
                for gg, kt in order:
                    gate_mm(gps_, gg,
                            h0T_[:, 8 * kt:8 * kt + 8],
                            whh0_sb[:, kt * G4 + gg * H:kt * G4 + (gg + 1) * H],
                            kt == 0, False)
                    if has_b0 and kt == 3:
                        gate_mm(gps_, gg, ones18[:],
                                b0_sb[:, gg * H:(gg + 1) * H], False, False)
                if decT_ is not None:
                    for gg, kt in order:
                        gate_mm(gps_, gg,
                                decT_[:, 8 * kt:8 * kt + 8],
                                wih0_sb[:, kt * G4 + gg * H:
                                        kt * G4 + (gg + 1) * H],
                                False, False)

            def emit_gates1_h(gps_, h1T_):
                order = ([(gg, kt) for gg in range(4) for kt in range(4)]
                         if SIM_SEQ else
                         [(gg, kt) for kt in range(4) for gg in range(4)])
                for gg, kt in order:
                    gate_mm(gps_, gg,
                            h1T_[:, 8 * kt:8 * kt + 8],
                            whh1_sb[:, kt * G4 + gg * H:kt * G4 + (gg + 1) * H],
                            kt == 0, False)
                    if has_b1 and kt == 3:
                        gate_mm(gps_, gg, ones18[:],
                                b1_sb[:, gg * H:(gg + 1) * H], False, False)

            def lstm_elementwise(g_ps, g_tag, cT, h_tag):
                """Gates PSUM [128(4 gate-groups at 32*gg),512] -> transposed
                [128,128] -> one tanh(0.5*x) ACT (g-gate weights pre-doubled,
                gate order i,f,o,g) -> h^T [128,32] bf16 (stationary form)."""
                g_sb = wk2.tile([P, S], BF16, tag=g_tag)
                if SIM_SEQ:
                    nc.vector.memset(g_sb[:], 0.0)
                    for gg in range(4):
                        nc.vector.tensor_copy(g_sb[32 * gg:32 * gg + 8, :],
                                              g_ps[32 * gg:32 * gg + 8, :])
                else:
                    nc.vector.tensor_copy(g_sb[:], g_ps[:])
                # per h-tile selector matmul (contiguous out): col layout
                # kt*32 + gg*8 + b ; out[m, 8gg+b] = g_sb[32gg+b, kt*128+m]
                gT_ps = psS.tile([P, P], F32, tag="psS")
                for kt in range(4):
                    nc.tensor.matmul(gT_ps[:, kt * 32:(kt + 1) * 32],
                                     g_sb[:, kt * P:(kt + 1) * P],
                                     selT_sb[:], start=True, stop=True)
                t_sb = wk.tile([P, P], BF16, tag="t_" + g_tag)
                nc.scalar.activation(t_sb[:], gT_ps[:], AF.Tanh, scale=0.5)
                # t cols: (kt, gate, b) with gate order i,f,o,g
                t_v = t_sb[:].rearrange("p (k g b) -> p k g b", k=4, g=4, b=8)
                s_sb = wk.tile([P, 96], BF16, tag="s_" + g_tag)
                s_v = s_sb[:].rearrange("p (k g b) -> p k g b", k=4, g=3, b=8)
                nc.vector.tensor_scalar(s_v[:], t_v[:, :, 0:3, :], 0.5, 0.5,
                                        op0=OP.mult, op1=OP.add)
                cT_v = cT[:].rearrange("p (k b) -> p k b", k=4)
                t1 = wk.tile([P, 32], F32, tag="t1")
                t1_v = t1[:].rearrange("p (k b) -> p k b", k=4)
                nc.vector.tensor_mul(t1_v[:], s_v[:, :, 0, :],
                                     t_v[:, :, 3, :])
                t2 = wk.tile([P, 32], F32, tag="t2")
                t2_v = t2[:].rearrange("p (k b) -> p k b", k=4)
                nc.vector.tensor_mul(t2_v[:], s_v[:, :, 1, :], cT_v[:])
                nc.vector.tensor_add(cT[:], t2[:], t1[:])
                nc.tensor.ldweights(ident[:])   # HAM keep-warm
                tc2 = wk.tile([P, 32], BF16, tag="tc_" + g_tag)
                nc.scalar.activation(tc2[:], cT[:], AF.Tanh)
                hT = htp.tile([P, 32], BF16, tag=h_tag)
                hT_v = hT[:].rearrange("p (k b) -> p k b", k=4)
                tc2_v = tc2[:].rearrange("p (k b) -> p k b", k=4)
                nc.vector.tensor_mul(hT_v[:], s_v[:, :, 2, :], tc2_v[:])
                return hT

            # ---------------- the recurrence ----------------
            for t in range(T):
                # early PE fodder: all gate matmuls that only need step-entry
                # state (they run while ACT chews the energy tanh). In SIM_SEQ
                # mode brackets must be contiguous, so these move to the end.
                g0_ps = psB.tile([P, S], F32, tag="psB", name="g0")
                g1_ps = psB.tile([P, S], F32, tag="psB", name="g1")
                if not SIM_SEQ:
                    emit_gates0_hdec(g0_ps, h0T, h1T if t > 0 else None)
                    emit_gates1_h(g1_ps, h1T)

                # ---- attention: energy tanh + scores ----
                # scores stay unpacked: the 32 serial matmuls hide under the
                # tanh chain, and the softmax can then read PSUM directly.
                scores_ps = psB.tile([BL, S], F32, tag="psB", name="sc")
                for ht in range(4):
                    e_t = epool.tile([P, BL * S], BF16, tag="e")
                    for b in range(BL):
                        eng = nc.gpsimd if (GPSIMD_QADD and b >= 5) else nc.vector
                        eng.tensor_scalar(
                            e_t[:, b * S:(b + 1) * S],
                            ep_sb[:, (ht * BL + b) * S:(ht * BL + b + 1) * S],
                            qbaT[:, 8 * ht + b:8 * ht + b + 1], None,
                            op0=OP.add)
                    if ht == 0:
                        # first tanh of the step is on the critical path:
                        # split it so it starts after only 4 q-adds
                        nc.scalar.activation(e_t[:, 0:4 * S], e_t[:, 0:4 * S],
                                             AF.Tanh)
                        nc.scalar.activation(e_t[:, 4 * S:], e_t[:, 4 * S:],
                                             AF.Tanh)
                    else:
                        nc.scalar.activation(e_t[:], e_t[:], AF.Tanh)
                    for b in range(BL):
                        kt = ht * 8 + b
                        nc.tensor.matmul(
                            scores_ps[:],
                            vblk_sb[:, 8 * kt:8 * kt + 8],
                            e_t[:, b * S:(b + 1) * S],
                            start=(kt == 0), stop=(kt == 31))

                # ---- softmax (reads scores PSUM directly) ----
                w_e = wk.tile([BL, S], BF16, tag="w_e")
                zsum = wk.tile([BL, 1], F32, tag="zsum")
                nc.scalar.activation(w_e[:], scores_ps[:], AF.Exp,
                                     accum_out=zsum[:])
                zrec = wk.tile([BL, 1], F32, tag="zrec")
                nc.vector.reciprocal(zrec[:], zsum[:])
                nc.tensor.ldweights(ident[:])   # HAM keep-warm
                w_n = wk.tile([BL, S], BF16, tag="w_n")
                nc.vector.tensor_scalar(w_n[:], w_e[:], zrec[:, 0:1], None,
                                        op0=OP.mult)
                if DEBUG_T0 and t == 0:
                    dump("scf", scores_f[:])
                    dump("wn", w_n[:])

                # ---- w transpose -> block-diag stationary ----
                wT_ps = psS.tile([P, 32], BF16, tag="psS")
                for j in range(4):
                    nc.tensor.matmul(wT_ps[:, 8 * j:8 * j + 8],
                                     w_n[:, 128 * j:128 * (j + 1)],
                                     ident[:8, :8], is_transpose=True)
                wblk_v = wblk_sb[:, 0:264].rearrange("p (b c) -> p b c",
                                                      c=33)
                wT_v = wT_ps[:].rearrange("p (s b) -> p b s", b=8)
                nc.vector.tensor_copy(wblk_v[:, :, 0:25:8], wT_v[:])

                # ---- context (col-packed by batch-pair) ----
                ctx_ps = psB.tile([P, S], F32, tag="psB", name="ctx")
                if SIM_SEQ:
                    order = [(g, r) for g in range(4) for r in range(8)]
                else:
                    order = [(g, r) for r in range(8) for g in range(4)]
                for g, r in order:
                    b = 2 * g + r // 4
                    st = r % 4
                    kt = b * 4 + st
                    first = (r == 0)
                    last = (r == 7)
                    nc.tensor.matmul(
                        ctx_ps[32 * g:32 * g + 8, :],
                        wblk_sb[:, 8 * kt:8 * kt + 8],
                        enc_sb[:, kt * S:(kt + 1) * S],
                        start=first, stop=last,
                        tile_position=(0, 32 * g),
                        skip_group_check=not SIM_SEQ)
                ctx_sb = wk2.tile([P, S], BF16, tag="ctx_sb")
                if SIM_SEQ:
                    nc.vector.memset(ctx_sb[:], 0.0)
                    for g in range(4):
                        nc.vector.tensor_copy(ctx_sb[32 * g:32 * g + 8, :],
                                              ctx_ps[32 * g:32 * g + 8, :])
                else:
                    nc.vector.tensor_copy(ctx_sb[:], ctx_ps[:])
                # transpose + sum the 4 partials: out[m, 8kt+b] =
                # sum_g ctx_sb[32g+b, kt*128+m]
                ctxT_ps = psS.tile([P, 32], F32, tag="psS")
                for kt in range(4):
                    nc.tensor.matmul(ctxT_ps[:, 8 * kt:8 * kt + 8],
                                     ctx_sb[:, kt * P:(kt + 1) * P],
                                     msk_sb[:], start=True, stop=True)
                ctxT_sb = wk2.tile([P, 32], BF16, tag="ctxT_sb")
                nc.vector.tensor_copy(ctxT_sb[:], ctxT_ps[:])
                if DEBUG_T0 and t == 0:
                    dump("ctxT", ctxT_sb[:])

                # ---- finish gates0: ctx part (koffset 4 in wih0) ----
                if SIM_SEQ:
                    def g0_bracket(gg):
                        for kt in range(4):
                            gate_mm(g0_ps, gg, h0T[:, 8 * kt:8 * kt + 8],
                                    whh0_sb[:, kt * G4 + gg * H:
                                            kt * G4 + (gg + 1) * H],
                                    kt == 0, False)
                        if has_b0:
                            gate_mm(g0_ps, gg, ones18[:],
                                    b0_sb[:, gg * H:(gg + 1) * H], False, False)
                        if t > 0:
                            for kt in range(4):
                                gate_mm(g0_ps, gg, h1T[:, 8 * kt:8 * kt + 8],
                                        wih0_sb[:, kt * G4 + gg * H:
                                                kt * G4 + (gg + 1) * H],
                                        False, False)
                        for kt in range(4):
                            gate_mm(g0_ps, gg, ctxT_sb[:, 8 * kt:8 * kt + 8],
                                    wih0_sb[:, (4 + kt) * G4 + gg * H:
                                            (4 + kt) * G4 + (gg + 1) * H],
                                    False, kt == 3)
                    for gg in range(4):
                        g0_bracket(gg)
                else:
                    order = [(gg, kt) for kt in range(4) for gg in range(4)]
                    for gg, kt in order:
                        gate_mm(g0_ps, gg, ctxT_sb[:, 8 * kt:8 * kt + 8],
                                wih0_sb[:, (4 + kt) * G4 + gg * H:
                                        (4 + kt) * G4 + (gg + 1) * H],
                                False, kt == 3)

                h0T = lstm_elementwise(g0_ps, "g0s", c0T, "h0T")
                if DEBUG_T0 and t == 0:
                    dump("h0T", h0T[:])
                    dump("c0T", c0T[:])

                # ---- LSTM layer 1: x part ----
                if SIM_SEQ:
                    for gg in range(4):
                        for kt in range(4):
                            gate_mm(g1_ps, gg, h1T[:, 8 * kt:8 * kt + 8],
                                    whh1_sb[:, kt * G4 + gg * H:
                                            kt * G4 + (gg + 1) * H],
                                    kt == 0, False)
                        if has_b1:
                            gate_mm(g1_ps, gg, ones18[:],
                                    b1_sb[:, gg * H:(gg + 1) * H], False, False)
                        for kt in range(4):
                            gate_mm(g1_ps, gg, h0T[:, 8 * kt:8 * kt + 8],
                                    wih1_sb[:, kt * G4 + gg * H:
                                            kt * G4 + (gg + 1) * H],
                                    False, kt == 3)
                else:
                    order = [(gg, kt) for kt in range(4) for gg in range(4)]
                    for gg, kt in order:
                        gate_mm(g1_ps, gg, h0T[:, 8 * kt:8 * kt + 8],
                                wih1_sb[:, kt * G4 + gg * H:
                                        kt * G4 + (gg + 1) * H],
                                False, kt == 3)
                h1T = lstm_elementwise(g1_ps, "g1s", c1T, "h1T")
                if DEBUG_T0 and t == 0:
                    dump("h1T", h1T[:])

                # ---- qba for next step first (unblocks next tanh ASAP) ----
                if t + 1 < T:
                    qbaT = emit_qpath(h1T)

                # ---- y = h1n @ Wf.T ----
                y_ps = psF.tile([BL, 1], F32, tag="ps8")
                for kt in range(4):
                    nc.tensor.matmul(y_ps[:], h1T[:, 8 * kt:8 * kt + 8],
                                     wf_sb[:, kt:kt + 1],
                                     start=(kt == 0), stop=(kt == 3))
                nc.vector.tensor_copy(y_acc[:, t:t + 1], y_ps[:])

            nc.sync.dma_start(out[:], y_acc[:])

    nc.compile()
    return nc


def _marshal(inputs):
    """Host-side shard + relayout. Returns (in_maps, flags)."""
    f32 = np.float32
    enc = np.asarray(inputs["encoder_outputs"], f32)
    Wa_h = np.asarray(inputs["Wa_h"], f32)
    Wa_e = np.asarray(inputs["Wa_e"], f32)
    ba = np.asarray(inputs["ba"], f32)
    v = np.asarray(inputs["v"], f32)
    W_ih_0 = np.asarray(inputs["W_ih_0"], f32)
    W_hh_0 = np.asarray(inputs["W_hh_0"], f32)
    b_0 = np.asarray(inputs["b_0"], f32)
    W_ih_1 = np.asarray(inputs["W_ih_1"], f32)
    W_hh_1 = np.asarray(inputs["W_hh_1"], f32)
    b_1 = np.asarray(inputs["b_1"], f32)
    Wf = np.asarray(inputs["Wf"], f32)

    has_ba = bool(np.any(ba != 0))
    has_b0 = bool(np.any(b_0 != 0))
    has_b1 = bool(np.any(b_1 != 0))

    def gate_perm(W):
        # torch gate order i,f,g,o -> i,f,o,2*g (one tanh(0.5*x) ACT serves
        # sigmoid-as-tanh for i,f,o and plain tanh for the doubled g rows)
        return np.concatenate([W[0:H], W[H:2 * H], W[3 * H:4 * H],
                               2.0 * W[2 * H:3 * H]], axis=0)

    W_ih_0 = gate_perm(W_ih_0)
    W_hh_0 = gate_perm(W_hh_0)
    W_ih_1 = gate_perm(W_ih_1)
    W_hh_1 = gate_perm(W_hh_1)
    b_0 = gate_perm(b_0.reshape(G4, 1)).reshape(G4)
    b_1 = gate_perm(b_1.reshape(G4, 1)).reshape(G4)

    def to_kxn(W, n_kt):  # W [N, K] -> [128, n_kt*N] bf16 layout (p,(kt,n))
        Wt = W.T.astype(nbf16)                       # [K, N]
        return np.ascontiguousarray(
            Wt.reshape(n_kt, P, W.shape[0]).transpose(1, 0, 2).reshape(P, -1))

    def to_T32(x, dt):  # [8, 512] -> [128, 32] col kt*8+b
        return np.ascontiguousarray(
            x.T.reshape(4, P, BL).transpose(1, 0, 2).reshape(P, 32).astype(dt))

    wih0T = to_kxn(W_ih_0, 8)
    whh0T = to_kxn(W_hh_0, 4)
    wih1T = to_kxn(W_ih_1, 4)
    whh1T = to_kxn(W_hh_1, 4)
    wahT = to_kxn(Wa_h, 4)
    waeT = to_kxn(Wa_e, 4)
    wfT = np.ascontiguousarray(Wf.reshape(4, P).T.astype(nbf16))  # [128,4]

    vblk = np.zeros((P, 256), nbf16)
    for ht in range(4):
        for b in range(BL):
            kt = ht * 8 + b
            vblk[:, 8 * kt + b] = v[ht * P:(ht + 1) * P].astype(nbf16)

    mskC = np.zeros((P, 8), nbf16)
    selT = np.zeros((P, 32), nbf16)
    for g in range(4):
        for b in range(BL):
            mskC[32 * g + b, b] = 1.0
            selT[32 * g + b, 8 * g + b] = 1.0

    shared = dict(wih0T=wih0T, whh0T=whh0T, wih1T=wih1T, whh1T=whh1T,
                  wahT=wahT, waeT=waeT, wfT=wfT, vblk=vblk,
                  mskC=mskC, selT=selT)
    if has_ba:
        shared["baRow"] = ba.reshape(1, H).astype(nbf16)
    if has_b0:
        shared["b0Row"] = b_0.reshape(1, G4).astype(nbf16)
    if has_b1:
        shared["b1Row"] = b_1.reshape(1, G4).astype(nbf16)

    enc_bf = enc.astype(nbf16)
    in_maps = []
    for c in range(N_CORES):
        sl = slice(c * BL, (c + 1) * BL)
        eb = enc_bf[sl]                                   # [8, 512, 512]
        enc_s = np.ascontiguousarray(
            eb.reshape(BL, 4, P, H).transpose(2, 0, 1, 3).reshape(P, -1))
        enc_t = np.ascontiguousarray(
            eb.transpose(0, 2, 1).reshape(BL, 4, P, S).reshape(BL * 4, P, S))
        m = dict(shared)
        m.update(
            enc_s=enc_s, enc_t=enc_t,
            h00T=to_T32(np.asarray(inputs["h0_0"], f32)[sl], nbf16),
            c00T=to_T32(np.asarray(inputs["c0_0"], f32)[sl], f32),
            h01T=to_T32(np.asarray(inputs["h0_1"], f32)[sl], nbf16),
            c01T=to_T32(np.asarray(inputs["c0_1"], f32)[sl], f32),
        )
        in_maps.append(m)
    return in_maps, (has_b0, has_b1, has_ba)


def kernel(**inputs):
    global _last_result
    in_maps, flags = _marshal(inputs)
    if flags not in _cache:
        _cache[flags] = build(*flags)
    nc = _cache[flags]
    res = run_bass_kernel_spmd(nc, in_maps, core_ids=list(range(N_CORES)))
    _last_result = res
    ys = np.concatenate([np.asarray(res.results[i]["out"], np.float32)
                         for i in range(N_CORES)], axis=0)   # [64, 64]
    bf_ = np.asarray(inputs["bf"], np.float32).reshape(1, 1)
    y = ys + bf_
    return y.reshape(B, T, 1).astype(np.float32)


if __name__ == "__main__":
    rng = np.random.default_rng(0)
    fake = {
        "encoder_outputs": rng.normal(size=(B, S, H)).astype(np.float32),
        "h0_0": rng.normal(size=(B, H)).astype(np.float32),
        "c0_0": rng.normal(size=(B, H)).astype(np.float32),
        "h0_1": rng.normal(size=(B, H)).astype(np.float32),
        "c0_1": rng.normal(size=(B, H)).astype(np.float32),
        "Wa_h": (rng.normal(size=(H, H)) * 0.05).astype(np.float32),
        "Wa_e": (rng.normal(size=(H, H)) * 0.05).astype(np.float32),
        "ba": np.zeros(H, np.float32),
        "v": (rng.normal(size=H) * 0.05).astype(np.float32),
        "W_ih_0": (rng.normal(size=(G4, 2 * H)) * 0.05).astype(np.float32),
        "W_hh_0": (rng.normal(size=(G4, H)) * 0.05).astype(np.float32),
        "b_0": np.zeros(G4, np.float32),
        "W_ih_1": (rng.normal(size=(G4, H)) * 0.05).astype(np.float32),
        "W_hh_1": (rng.normal(size=(G4, H)) * 0.05).astype(np.float32),
        "b_1": np.zeros(G4, np.float32),
        "Wf": (rng.normal(size=(1, H)) * 0.05).astype(np.float32),
        "bf": np.zeros(1, np.float32),
    }
    y = kernel(**fake)
    print("kernel output", y.shape, y.dtype, float(np.abs(y).max()))
